# revision 1
# baseline (speedup 1.0000x reference)
"""GNN message-passing (2x GAT + 2x GIN, 2 edge types) on 8 trn2 NeuronCores.

Sharding: cores 0-3 handle edge type 0, cores 4-7 edge type 1 (independent
quads). Within a quad, nodes are sharded by dst range (12500/core, padded to
12544); each edge lives on the core owning its dst, bucketed by src-quarter so
gather indices are quarter-local (< 32768, fits int16).

Per GAT layer: each core computes z/el/er for its node shard on PE (el/er come
free as extra matmul columns using host-precomputed W@al / W@ar), packs
[z|el|er] into bf16 rows, AllGathers within the quad, then per edge chunk:
dma_gather of packed src rows + dma_gather of (el,er) dst rows, edge softmax
without segment-max (logits are O(1) so exp is safe; exp(e)/sum(exp(e)) is
mathematically identical to the max-subtracted form), dma_scatter_add of
[w*z | w] f32 into the local dst accumulator, then normalize num/den locally.

GIN layers: bf16 gathers/scatter-adds of neighbor rows, feature-major MLP on
PE, batchnorm stats per-partition + a tiny quad AllReduce.

Scatter accumulators are ExternalOutputs so the runtime pre-zeroes them.
"""

import sys

for _p in ("/opt/trn_rl_repo",):
    if _p not in sys.path:
        sys.path.insert(0, _p)

import numpy as np
import ml_dtypes

import concourse.bacc as bacc
import concourse.bass as bass
import concourse.tile as tile
import concourse.mybir as mybir
from concourse.bass_utils import run_bass_kernel_spmd

FP32 = mybir.dt.float32
BF16 = mybir.dt.bfloat16
I16 = mybir.dt.int16
AF = mybir.ActivationFunctionType
ALU = mybir.AluOpType

# problem constants
N, IN, HID, H, D = 50000, 128, 256, 4, 64
E, T = 400000, 2
BN_EPS = 1e-5
P = 4                     # cores per quad
NQ = N // P               # real nodes per core: 12500
NCP = 12544               # padded (98 * 128); dump row = NQ
NT = NCP // 128           # 98 node tiles
ZW = 384                  # packed zel row: [z 256 | el 4 | er 4 | pad]
CE = 1024                 # edge-chunk tokens (small: SWDGE ring safety)
SLOTS = CE // 128         # 8
ACC_W = 320               # GAT accumulator row: [num 256 | den 4 | junk]
RGROUPS = [[0, 1, 2, 3], [4, 5, 6, 7]]
DEBUG_TAPS = False
EDGE_DBG = 0  # 1=no scatter, 2=+no er-gather, 3=+no exp/msg
STAGES = 99   # bisect knob: 1=gat_node0 2=+AG 3=+edge 4=+post 5=+gat1 6=+gin0 7=all


def _bf(x):
    return np.asarray(x, dtype=ml_dtypes.bfloat16)


def _wrap_idx(a):
    """[CE] ints -> [128, CE//16] int16 SWDGE wrapped layout (token i at
    [i % 16, i // 16], replicated across the 8 Q7 cores)."""
    w = a.reshape(CE // 16, 16).T.astype(np.int16)
    return np.tile(w, (8, 1))


def _rank_sort(ss, dd):
    """Sort edges by (rank-within-dst, dst). Within a rank slice every dst is
    unique, which makes dma_scatter_add race-free per instruction."""
    order = np.argsort(dd, kind="stable")
    ds = dd[order]
    n = len(ds)
    if n == 0:
        return ss[:0], dd[:0], np.zeros(0, np.int64)
    first = np.r_[True, ds[1:] != ds[:-1]]
    idx_first = np.maximum.accumulate(np.where(first, np.arange(n), 0))
    rank = np.arange(n) - idx_first
    order2 = np.argsort(rank, kind="stable")
    perm = order[order2]
    return ss[perm], dd[perm], rank[order2]


def _preprocess(inputs):
    feats = np.asarray(inputs["feats"], np.float32)
    edges = [
        (np.asarray(inputs["src0"]), np.asarray(inputs["dst0"])),
        (np.asarray(inputs["src1"]), np.asarray(inputs["dst1"])),
    ]

    buckets = {}
    slice_cnt = {}   # (q,r,b) -> per-rank counts
    for q in range(T):
        src, dst = edges[q]
        for r in range(P):
            m = (dst >= r * NQ) & (dst < (r + 1) * NQ)
            es, ed = src[m], dst[m] - r * NQ
            for b in range(P):
                sel = (es >= b * NQ) & (es < (b + 1) * NQ)
                ss = (es[sel] - b * NQ).astype(np.int64)
                dd = ed[sel].astype(np.int64)
                ss, dd, rank = _rank_sort(ss, dd)
                buckets[(q, r, b)] = (ss, dd, rank)
                slice_cnt[(q, r, b)] = np.bincount(rank) if len(rank) else \
                    np.zeros(1, np.int64)

    # shared chunk plan: per bucket, rank-slice sizes = max over all cores,
    # padded to 128; slices chopped into chunks of <= CE tokens.
    chunk_plan = []         # list of (bucket, n_tokens)
    slice_max = {}          # b -> padded per-rank sizes
    for b in range(P):
        nr = max(len(slice_cnt[(q, r, b)]) for q in range(T) for r in range(P))
        sm = np.zeros(nr, np.int64)
        for q in range(T):
            for r in range(P):
                cc = slice_cnt[(q, r, b)]
                sm[: len(cc)] = np.maximum(sm[: len(cc)], cc)
        sm = ((sm + 127) // 128) * 128
        slice_max[b] = sm
        for srank in sm:
            left = int(srank)
            while left > 0:
                take = min(left, CE)
                chunk_plan.append((b, take))
                left -= take
    nch = len(chunk_plan)

    feats_bf = _bf(feats)

    in_maps = []
    for c in range(8):
        q, r = c // P, c % P
        sidx = np.zeros((nch, 128, CE // 16), np.int16)
        didx = np.zeros((nch, 128, CE // 16), np.int16)
        # build each bucket's padded token stream, then chop by chunk_plan
        streams = {}
        for b in range(P):
            ss, dd, rank = buckets[(q, r, b)]
            sm = slice_max[b]
            tot = int(sm.sum())
            sp = np.zeros(tot, np.int64)
            dp = np.full(tot, NQ, np.int64)      # pad -> dump row
            off = 0
            pos = 0
            for rr, srank in enumerate(sm):
                ncr = int(np.sum(rank == rr))
                sp[off:off + ncr] = ss[pos:pos + ncr]
                dp[off:off + ncr] = dd[pos:pos + ncr]
                pos += ncr
                off += int(srank)
            streams[b] = (sp, dp)
        cursor = {b: 0 for b in range(P)}
        for k, (b, ntok) in enumerate(chunk_plan):
            sp, dp = streams[b]
            cu = cursor[b]
            spc = np.zeros(CE, np.int64)
            dpc = np.full(CE, NQ, np.int64)
            spc[:ntok] = sp[cu:cu + ntok]
            dpc[:ntok] = dp[cu:cu + ntok]
            cursor[b] = cu + ntok
            sidx[k] = _wrap_idx(spc)
            didx[k] = _wrap_idx(dpc)

        feats_loc = np.zeros((NCP, IN), np.float32)
        feats_loc[:NQ] = feats[r * NQ:(r + 1) * NQ]

        def gat_wx(Wt, al, ar):
            Wr = Wt.reshape(Wt.shape[0], H, D)
            wal = np.einsum("khd,hd->kh", Wr, al)
            war = np.einsum("khd,hd->kh", Wr, ar)
            wx = np.concatenate([Wt, wal, war], 1)          # [F_in, 264]
            kc = wx.shape[0] // 128
            return _bf(np.ascontiguousarray(
                wx.reshape(kc, 128, 264).transpose(1, 0, 2)))

        def wchunks(Wt):
            kc = Wt.shape[0] // 128
            return _bf(np.ascontiguousarray(
                Wt.reshape(kc, 128, Wt.shape[1]).transpose(1, 0, 2)))

        def fvec(v):
            # [256] feature vector -> [128, 2, 1]  (feature = half*128 + p)
            return np.ascontiguousarray(
                np.asarray(v, np.float32).reshape(2, 128)
                .transpose(1, 0)[:, :, None])

        g = lambda k: np.asarray(inputs[k], np.float32)

        m = {
            "feats_g": feats_bf,
            "feats_loc": _bf(feats_loc),
            "sidx": sidx,
            "didx": didx,
            "w0x": gat_wx(g("gat0_W")[q], g("gat0_al")[q], g("gat0_ar")[q]),
            "w1x": gat_wx(g("gat1_W")[q], g("gat1_al")[q], g("gat1_ar")[q]),
            "b0": np.tile(g("gat0_b")[q][None, :], (128, 1)).astype(np.float32),
            "b1": np.tile(g("gat1_b")[q][None, :], (128, 1)).astype(np.float32),
            "g0w1": wchunks(g("gin0_W1")[q]),
            "g0w2": wchunks(g("gin0_W2")[q]),
            "g0b1": fvec(g("gin0_b1")[q]),
            "g0g1": fvec(g("gin0_g1")[q]),
            "g0be1": fvec(g("gin0_be1")[q]),
            "g0b2": fvec(g("gin0_b2")[q]),
            "g1w1": wchunks(g("gin1_W1")[q]),
            "g1w2": wchunks(g("gin1_W2")[q]),
            "g1b1": fvec(g("gin1_b1")[q]),
            "g1g1": fvec(g("gin1_g1")[q]),
            "g1be1": fvec(g("gin1_be1")[q]),
            "g1b2": fvec(g("gin1_b2")[q]),
            "eps0": np.full((128, 1), 1.0 + float(g("gin0_eps")[q]), np.float32),
            "eps1": np.full((128, 1), 1.0 + float(g("gin1_eps")[q]), np.float32),
            "identity": _bf(np.eye(128)),
        }
        in_maps.append(m)
    return in_maps, tuple(chunk_plan)


def _rows(dram, r0, nt, width):
    """rows [r0*128, (r0+nt)*128) of a [*, width] DRAM tensor as [128, nt, w]."""
    return dram[r0 * 128:(r0 + nt) * 128, :].rearrange("(t p) f -> p t f", p=128)


def build_program(chunk_plan):
    nc = bacc.Bacc("TRN2", target_bir_lowering=False, debug=False,
                   num_devices=8)

    dp = nc.declare_dram_parameter
    feats_g = dp("feats_g", [N, IN], BF16, isOutput=False)
    feats_loc = dp("feats_loc", [NCP, IN], BF16, isOutput=False)
    nch = len(chunk_plan)
    sidx_d = dp("sidx", [nch, 128, CE // 16], I16, isOutput=False)
    didx_d = dp("didx", [nch, 128, CE // 16], I16, isOutput=False)
    w0x_d = dp("w0x", [128, 1, 264], BF16, isOutput=False)
    w1x_d = dp("w1x", [128, 2, 264], BF16, isOutput=False)
    b0_d = dp("b0", [128, HID], FP32, isOutput=False)
    b1_d = dp("b1", [128, HID], FP32, isOutput=False)
    g0w1_d = dp("g0w1", [128, 3, HID], BF16, isOutput=False)
    g0w2_d = dp("g0w2", [128, 2, HID], BF16, isOutput=False)
    g1w1_d = dp("g1w1", [128, 2, HID], BF16, isOutput=False)
    g1w2_d = dp("g1w2", [128, 2, HID], BF16, isOutput=False)
    vec_d = {}
    for nm in ("g0b1", "g0g1", "g0be1", "g0b2",
               "g1b1", "g1g1", "g1be1", "g1b2"):
        vec_d[nm] = dp(nm, [128, 2, 1], FP32, isOutput=False)
    eps0_d = dp("eps0", [128, 1], FP32, isOutput=False)
    eps1_d = dp("eps1", [128, 1], FP32, isOutput=False)
    ident_d = dp("identity", [128, 128], BF16, isOutput=False)

    out_d = dp("out", [NCP, HID], FP32, isOutput=True)
    # scatter accumulators; ExternalOutputs are pre-zeroed by the runtime
    accg = [dp("accg0", [NCP, ACC_W], FP32, isOutput=True),
            dp("accg1", [NCP, ACC_W], FP32, isOutput=True)]
    accn = [dp("accn0", [NCP, HID + IN], BF16, isOutput=True),
            dp("accn1", [NCP, HID], BF16, isOutput=True)]

    # DRAM scratch
    zel_loc = nc.dram_tensor("zel_loc", [NCP, ZW], BF16)
    zel_full = nc.dram_tensor("zel_full", [P * NCP, ZW], BF16)
    hq_loc = nc.dram_tensor("hq_loc", [NCP, HID], BF16)
    hq_full = nc.dram_tensor("hq_full", [P * NCP, HID], BF16)
    arb_in = [nc.dram_tensor(f"arb_in{i}", [128, 4], FP32) for i in range(2)]
    arb_out = [nc.dram_tensor(f"arb_out{i}", [128, 4], FP32) for i in range(2)]
    if DEBUG_TAPS:
        dbg_xcat = nc.dram_tensor("dbg_xcat", [NCP, HID + IN], BF16)
        dbg_x1T = nc.dram_tensor("dbg_x1T", [128, 2 * NCP], BF16)
        dbg_stats = nc.dram_tensor("dbg_stats", [128, 16], FP32)

    with tile.TileContext(nc) as tc:
        cst = tc.alloc_tile_pool(name="cst", bufs=1)

        def ld(dram, shape, dtype):
            t = cst.tile(shape, dtype, tag=dram.name + "_sb")
            nc.sync.dma_start(out=t[:], in_=dram[tuple(slice(None) for _ in shape)])
            return t

        ident = ld(ident_d, [128, 128], BF16)
        w0x = ld(w0x_d, [128, 1, 264], BF16)
        w1x = ld(w1x_d, [128, 2, 264], BF16)
        b0 = ld(b0_d, [128, HID], FP32)
        b1 = ld(b1_d, [128, HID], FP32)
        g0w1 = ld(g0w1_d, [128, 3, HID], BF16)
        g0w2 = ld(g0w2_d, [128, 2, HID], BF16)
        g1w1 = ld(g1w1_d, [128, 2, HID], BF16)
        g1w2 = ld(g1w2_d, [128, 2, HID], BF16)
        vec = {nm: ld(d, [128, 2, 1], FP32) for nm, d in vec_d.items()}
        eps0 = ld(eps0_d, [128, 1], FP32)
        eps1 = ld(eps1_d, [128, 1], FP32)

        # ---------------- GAT node phase ----------------
        def gat_node(src_dram, f_in, wx):
            kc = f_in // 128
            with tc.tile_pool(name="gn", bufs=3) as pool, \
                 tc.tile_pool(name="gnp", bufs=2, space="PSUM") as pp:
                for c0 in range(0, NT, 4):
                    nt = min(4, NT - c0)
                    hsrc = pool.tile([128, nt, f_in], BF16, tag="hsrc")
                    nc.sync.dma_start(out=hsrc[:], in_=_rows(src_dram, c0, nt, f_in))
                    hT = pool.tile([128, kc, nt, 128], BF16, tag="hT")
                    for t in range(nt):
                        for k2 in range(kc):
                            pt = pp.tile([128, 128], BF16, tag="tp")
                            nc.tensor.transpose(
                                out=pt[:], in_=hsrc[:, t, k2 * 128:(k2 + 1) * 128],
                                identity=ident[:])
                            nc.any.tensor_copy(out=hT[:, k2, t, :], in_=pt[:])
                    zel = pool.tile([128, nt, ZW], BF16, tag="zel")
                    nc.vector.memset(zel[:, :, 264:ZW], 0.0)
                    for t in range(nt):
                        zp = pp.tile([128, 264], FP32, tag="zp")
                        for k2 in range(kc):
                            nc.tensor.matmul(
                                zp[:], lhsT=hT[:, k2, t, :], rhs=wx[:, k2, :],
                                start=(k2 == 0), stop=(k2 == kc - 1))
                        nc.any.tensor_copy(out=zel[:, t, 0:264], in_=zp[:])
                    nc.sync.dma_start(out=_rows(zel_loc, c0, nt, ZW), in_=zel[:])

        # ---------------- GAT edge phase ----------------
        def gat_edge(acc):
            with tc.tile_pool(name="ge", bufs=2) as pool:
                for ci, (b, ntok) in enumerate(chunk_plan):
                    if EDGE_DBG >= 4 and b != 0:
                        continue
                    sl = ntok // 128
                    st = pool.tile([128, CE // 16], I16, tag="st")
                    nc.sync.dma_start(out=st[:], in_=sidx_d[ci, :, :])
                    dt_ = pool.tile([128, CE // 16], I16, tag="dt")
                    nc.sync.dma_start(out=dt_[:], in_=didx_d[ci, :, :])
                    zg = pool.tile([128, SLOTS, ZW], BF16, tag="zg")
                    nc.gpsimd.dma_gather(
                        zg[:, 0:sl, :], zel_full[b * NCP:(b + 1) * NCP, :],
                        st[:, 0:ntok // 16], ntok, ntok, ZW)
                    if EDGE_DBG >= 3:
                        continue
                    lg = pool.tile([128, SLOTS, H], FP32, tag="lg")
                    if EDGE_DBG < 2:
                        eg = pool.tile([128, SLOTS, ZW], BF16, tag="eg")
                        nc.gpsimd.dma_gather(
                            eg[:, 0:sl, :], zel_loc[:, :],
                            dt_[:, 0:ntok // 16], ntok, ntok, ZW)
                        nc.vector.tensor_tensor(
                            out=lg[:, 0:sl, :], in0=zg[:, 0:sl, 256:260],
                            in1=eg[:, 0:sl, 260:264], op=ALU.add)
                    else:
                        nc.vector.tensor_copy(out=lg[:, 0:sl, :],
                                              in_=zg[:, 0:sl, 256:260])
                    lr = pool.tile([128, SLOTS, H], FP32, tag="lr")
                    nc.vector.scalar_tensor_tensor(
                        out=lr[:, 0:sl, :], in0=lg[:, 0:sl, :], scalar=0.2,
                        in1=lg[:, 0:sl, :], op0=ALU.mult, op1=ALU.max)
                    w_ = pool.tile([128, SLOTS, H], FP32, tag="w")
                    nc.scalar.activation(out=w_[:, 0:sl, :],
                                         in_=lr[:, 0:sl, :], func=AF.Exp)
                    msg = pool.tile([128, SLOTS, ACC_W], FP32, tag="msg")
                    nc.vector.memset(msg[:, 0:sl, 260:ACC_W], 0.0)
                    nc.vector.tensor_tensor(
                        out=msg[:, 0:sl, 0:256].rearrange(
                            "p s (h d) -> p s h d", h=H),
                        in0=zg[:, 0:sl, 0:256].rearrange(
                            "p s (h d) -> p s h d", h=H),
                        in1=w_[:, 0:sl, :].unsqueeze(3).broadcast_to(
                            [128, sl, H, D]),
                        op=ALU.mult)
                    nc.vector.tensor_copy(out=msg[:, 0:sl, 256:260],
                                          in_=w_[:, 0:sl, :])
                    if EDGE_DBG < 1:
                        nc.gpsimd.dma_scatter_add(
                            acc[:, :], msg[:, 0:sl, :], dt_[:, 0:ntok // 16],
                            ntok, ntok, ACC_W)

        # ---------------- GAT post (normalize + bias + relu) ----------------
        def gat_post(acc, bias, dst_dram):
            with tc.tile_pool(name="gp", bufs=3) as pool:
                for c0 in range(0, NT, 4):
                    nt = min(4, NT - c0)
                    a = pool.tile([128, nt, 260], FP32, tag="ac")
                    nc.sync.dma_start(
                        out=a[:],
                        in_=acc[c0 * 128:(c0 + nt) * 128, 0:260].rearrange(
                            "(t p) f -> p t f", p=128))
                    dmax = pool.tile([128, nt, H], FP32, tag="dmax")
                    nc.vector.tensor_scalar_max(dmax[:], a[:, :, 256:260], 1e-9)
                    rec = pool.tile([128, nt, H], FP32, tag="rec")
                    nc.vector.reciprocal(rec[:], dmax[:])
                    hb = pool.tile([128, nt, HID], FP32, tag="hb")
                    nc.vector.tensor_tensor(
                        out=hb[:].rearrange("p s (h d) -> p s h d", h=H),
                        in0=a[:, :, 0:256].rearrange("p s (h d) -> p s h d", h=H),
                        in1=rec[:].unsqueeze(3).broadcast_to([128, nt, H, D]),
                        op=ALU.mult)
                    hb2 = pool.tile([128, nt, HID], FP32, tag="hb2")
                    nc.vector.tensor_tensor(
                        out=hb2[:], in0=hb[:],
                        in1=bias[:].unsqueeze(1).broadcast_to([128, nt, HID]),
                        op=ALU.add)
                    ht = pool.tile([128, nt, HID], BF16, tag="ht")
                    nc.scalar.activation(out=ht[:], in_=hb2[:], func=AF.Relu)
                    nc.sync.dma_start(out=_rows(dst_dram, c0, nt, HID), in_=ht[:])

        # ---------------- GIN edge phase ----------------
        def gin_edge(gin):
            acc = accn[gin]
            step = (HID + IN) if gin == 0 else HID
            with tc.tile_pool(name="ne", bufs=2) as pool:
                for ci, (b, ntok) in enumerate(chunk_plan):
                    if EDGE_DBG >= 4 and b != 0:
                        continue
                    sl = ntok // 128
                    st = pool.tile([128, CE // 16], I16, tag="st")
                    nc.sync.dma_start(out=st[:], in_=sidx_d[ci, :, :])
                    dt_ = pool.tile([128, CE // 16], I16, tag="dt")
                    nc.sync.dma_start(out=dt_[:], in_=didx_d[ci, :, :])
                    hg = pool.tile([128, SLOTS, HID], BF16, tag="hg")
                    nc.gpsimd.dma_gather(
                        hg[:, 0:sl, :], hq_full[b * NCP:(b + 1) * NCP, :],
                        st[:, 0:ntok // 16], ntok, ntok, HID)
                    nc.gpsimd.dma_scatter_add(
                        acc[:, 0:HID], hg[:, 0:sl, :], dt_[:, 0:ntok // 16],
                        ntok, ntok, HID, elem_step=step)
                    if gin == 0:
                        fg = pool.tile([128, SLOTS, IN], BF16, tag="fg")
                        nc.gpsimd.dma_gather(
                            fg[:, 0:sl, :], feats_g[b * NQ:(b + 1) * NQ, :],
                            st[:, 0:ntok // 16], ntok, ntok, IN)
                        nc.gpsimd.dma_scatter_add(
                            acc[:, HID:HID + IN], fg[:, 0:sl, :],
                            dt_[:, 0:ntok // 16], ntok, ntok, IN,
                            elem_step=step)

        # ---------------- GIN node phase ----------------
        def gin_node(gin, dst_dram, out_f32):
            acc = accn[gin]
            w_in = (HID + IN) if gin == 0 else HID
            kc = w_in // 128
            w1 = g0w1 if gin == 0 else g1w1
            w2 = g0w2 if gin == 0 else g1w2
            epsv = eps0 if gin == 0 else eps1
            pre = "g0" if gin == 0 else "g1"
            with tc.tile_pool(name="nn", bufs=3) as pool, \
                 tc.tile_pool(name="nnb", bufs=1) as big, \
                 tc.tile_pool(name="nnp", bufs=2, space="PSUM") as pp:
                x1T = big.tile([128, 2, NCP], BF16, tag="x1T")
                run_s = big.tile([128, 2, 1], FP32, tag="run_s")
                run_q = big.tile([128, 2, 1], FP32, tag="run_q")
                nc.vector.memset(run_s[:], 0.0)
                nc.vector.memset(run_q[:], 0.0)
                # pass A: x1^T = W1^T @ xcat^T (feature-major), plus stats
                for c0 in range(0, NT, 4):
                    nt = min(4, NT - c0)
                    a = pool.tile([128, nt, w_in], BF16, tag="a")
                    nc.sync.dma_start(out=a[:], in_=_rows(acc, c0, nt, w_in))
                    hs = pool.tile([128, nt, HID], BF16, tag="hs")
                    nc.sync.dma_start(out=hs[:], in_=_rows(hq_loc, c0, nt, HID))
                    xc = pool.tile([128, nt, w_in], BF16, tag="xc")
                    nc.vector.scalar_tensor_tensor(
                        out=xc[:, :, 0:HID], in0=hs[:], scalar=epsv[:],
                        in1=a[:, :, 0:HID], op0=ALU.mult, op1=ALU.add)
                    if gin == 0:
                        fs = pool.tile([128, nt, IN], BF16, tag="fs")
                        nc.sync.dma_start(out=fs[:],
                                          in_=_rows(feats_loc, c0, nt, IN))
                        nc.vector.scalar_tensor_tensor(
                            out=xc[:, :, HID:w_in], in0=fs[:], scalar=epsv[:],
                            in1=a[:, :, HID:w_in], op0=ALU.mult, op1=ALU.add)
                    if DEBUG_TAPS and gin == 0:
                        nc.sync.dma_start(
                            out=_rows(dbg_xcat, c0, nt, w_in), in_=xc[:])
                    xT = pool.tile([128, kc, nt, 128], BF16, tag="xT")
                    for t in range(nt):
                        for k2 in range(kc):
                            pt = pp.tile([128, 128], BF16, tag="tp2")
                            nc.tensor.transpose(
                                out=pt[:], in_=xc[:, t, k2 * 128:(k2 + 1) * 128],
                                identity=ident[:])
                            nc.any.tensor_copy(out=xT[:, k2, t, :], in_=pt[:])
                    for hf in range(2):
                        xp = pp.tile([128, 512], FP32, tag="x1p")
                        for k2 in range(kc):
                            nc.tensor.matmul(
                                xp[:, 0:nt * 128],
                                lhsT=w1[:, k2, hf * 128:(hf + 1) * 128],
                                rhs=xT[:, k2, :, :].rearrange(
                                    "p t f -> p (t f)"),
                                start=(k2 == 0), stop=(k2 == kc - 1))
                        # stats over REAL nodes only (exclude pad/dump rows)
                        real = min(nt * 128, max(0, NQ - c0 * 128))
                        if real > 0:
                            sq = pool.tile([128, 512], BF16, tag="sq")
                            sqa = pool.tile([128, 1], FP32, tag="sqa")
                            nc.scalar.activation(
                                out=sq[:, 0:real], in_=xp[:, 0:real],
                                func=AF.Square, accum_out=sqa[:])
                            sm = pool.tile([128, 1], FP32, tag="sm")
                            nc.vector.tensor_reduce(
                                out=sm[:], in_=xp[:, 0:real],
                                axis=mybir.AxisListType.X, op=ALU.add)
                            nc.vector.tensor_add(run_q[:, hf, :],
                                                 run_q[:, hf, :], sqa[:])
                            nc.vector.tensor_add(run_s[:, hf, :],
                                                 run_s[:, hf, :], sm[:])
                        nc.vector.tensor_copy(
                            out=x1T[:, hf, c0 * 128:(c0 + nt) * 128],
                            in_=xp[:, 0:nt * 128])
                if DEBUG_TAPS and gin == 0:
                    nc.sync.dma_start(
                        out=dbg_x1T[:, :],
                        in_=x1T[:].rearrange("p a b -> p (a b)"))
                # stats allreduce
                arp = pool.tile([128, 4], FP32, tag="arp")
                nc.vector.tensor_copy(out=arp[:, 0:2], in_=run_s[:, :, 0])
                nc.vector.tensor_copy(out=arp[:, 2:4], in_=run_q[:, :, 0])
                nc.sync.dma_start(out=arb_in[gin][:, :], in_=arp[:])
                nc.gpsimd.collective_compute(
                    "AllReduce", ALU.add, replica_groups=RGROUPS,
                    ins=[arb_in[gin][:, :].opt()],
                    outs=[arb_out[gin][:, :].opt()])
                art = pool.tile([128, 4], FP32, tag="art")
                nc.sync.dma_start(out=art[:], in_=arb_out[gin][:, :])
                mu = pool.tile([128, 2], FP32, tag="mu")
                nc.vector.tensor_scalar_mul(mu[:], art[:, 0:2], 1.0 / N)
                msq = pool.tile([128, 2], FP32, tag="msq")
                nc.vector.tensor_scalar_mul(msq[:], art[:, 2:4], 1.0 / N)
                mu2 = pool.tile([128, 2], FP32, tag="mu2")
                nc.vector.tensor_mul(mu2[:], mu[:], mu[:])
                var = pool.tile([128, 2], FP32, tag="var")
                nc.vector.tensor_sub(var[:], msq[:], mu2[:])
                vare = pool.tile([128, 2], FP32, tag="vare")
                nc.vector.tensor_scalar_add(vare[:], var[:], BN_EPS)
                sd = pool.tile([128, 2], FP32, tag="sd")
                nc.scalar.activation(out=sd[:], in_=vare[:], func=AF.Sqrt)
                rsd = pool.tile([128, 2], FP32, tag="rsd")
                nc.vector.reciprocal(rsd[:], sd[:])
                # bn: (x1 + b1 - (mu1 + b1)) * scale + be  -- b1 cancels
                scl = pool.tile([128, 2], FP32, tag="scl")
                nc.vector.tensor_mul(scl[:], rsd[:], vec[pre + "g1"][:, :, 0])
                mus = pool.tile([128, 2], FP32, tag="mus")
                nc.vector.tensor_mul(mus[:], mu[:], scl[:])
                shf = pool.tile([128, 2], FP32, tag="shf")
                nc.vector.tensor_sub(shf[:], vec[pre + "be1"][:, :, 0], mus[:])
                if DEBUG_TAPS and gin == 0:
                    dst_ = pool.tile([128, 16], FP32, tag="dbgst")
                    nc.vector.tensor_copy(dst_[:, 0:2], run_s[:, :, 0])
                    nc.vector.tensor_copy(dst_[:, 2:4], run_q[:, :, 0])
                    nc.vector.tensor_copy(dst_[:, 4:6], mu[:])
                    nc.vector.tensor_copy(dst_[:, 6:8], var[:])
                    nc.vector.tensor_copy(dst_[:, 8:10], scl[:])
                    nc.vector.tensor_copy(dst_[:, 10:12], shf[:])
                    nc.vector.tensor_copy(dst_[:, 12:14], art[:, 0:2])
                    nc.vector.tensor_copy(dst_[:, 14:16], art[:, 2:4])
                    nc.sync.dma_start(out=dbg_stats[:, :], in_=dst_[:])
                # pass B: bn+relu, second matmul, +b2, relu, transpose out
                for c0 in range(0, NT, 4):
                    nt = min(4, NT - c0)
                    x1n = pool.tile([128, 2, 512], BF16, tag="x1n")
                    for hf in range(2):
                        nc.scalar.activation(
                            out=x1n[:, hf, 0:nt * 128],
                            in_=x1T[:, hf, c0 * 128:(c0 + nt) * 128],
                            func=AF.Relu, scale=scl[:, hf:hf + 1],
                            bias=shf[:, hf:hf + 1])
                    ho = pool.tile([128, 2, 512], BF16, tag="ho")
                    for hf in range(2):
                        x2p = pp.tile([128, 512], FP32, tag="x2p")
                        for k2 in range(2):
                            nc.tensor.matmul(
                                x2p[:, 0:nt * 128],
                                lhsT=w2[:, k2, hf * 128:(hf + 1) * 128],
                                rhs=x1n[:, k2, 0:nt * 128],
                                start=(k2 == 0), stop=(k2 == 1))
                        nc.scalar.activation(
                            out=ho[:, hf, 0:nt * 128], in_=x2p[:, 0:nt * 128],
                            func=AF.Relu, bias=vec[pre + "b2"][:, hf, :])
                    hout = pool.tile([128, nt, HID],
                                     FP32 if out_f32 else BF16, tag="hout")
                    for t in range(nt):
                        for hf in range(2):
                            pt = pp.tile([128, 128], BF16, tag="tp2")
                            nc.tensor.transpose(
                                out=pt[:],
                                in_=ho[:, hf, t * 128:(t + 1) * 128],
                                identity=ident[:])
                            nc.any.tensor_copy(
                                out=hout[:, t, hf * 128:(hf + 1) * 128],
                                in_=pt[:])
                    nc.sync.dma_start(out=_rows(dst_dram, c0, nt, HID),
                                      in_=hout[:])

        def allgather(src, dstf):
            nc.gpsimd.collective_compute(
                "AllGather", ALU.bypass, replica_groups=RGROUPS,
                ins=[src[:, :].opt()], outs=[dstf[:, :].opt()])

        # ---------------- zero the scatter accumulators ----------------
        with tc.tile_pool(name="zz", bufs=1) as zp:
            for acc_t, wdt, dt_ in ((accg[0], ACC_W, FP32),
                                    (accg[1], ACC_W, FP32),
                                    (accn[0], HID + IN, BF16),
                                    (accn[1], HID, BF16)):
                zt = zp.tile([128, 8, wdt], dt_, tag="z_" + acc_t.name)
                nc.vector.memset(zt[:], 0.0)
                for r0 in range(0, NT, 8):
                    nt = min(8, NT - r0)
                    nc.sync.dma_start(out=_rows(acc_t, r0, nt, wdt),
                                      in_=zt[:, 0:nt, :])

        # ---------------- full schedule ----------------
        gat_node(feats_loc, IN, w0x)
        if STAGES >= 2:
            allgather(zel_loc, zel_full)
        if STAGES >= 3:
            gat_edge(accg[0])
        if STAGES >= 4:
            gat_post(accg[0], b0, hq_loc)
        if STAGES >= 5:
            gat_node(hq_loc, HID, w1x)
            allgather(zel_loc, zel_full)
            gat_edge(accg[1])
            gat_post(accg[1], b1, hq_loc)
        if STAGES >= 6:
            allgather(hq_loc, hq_full)
            gin_edge(0)
            gin_node(0, hq_loc, out_f32=False)
        if STAGES >= 7:
            allgather(hq_loc, hq_full)
            gin_edge(1)
            gin_node(1, out_d, out_f32=True)

        cst.release()

    nc.compile()
    return nc


_CACHE = {}


def kernel(**inputs):
    in_maps, chunk_plan = _preprocess(inputs)
    nc = _CACHE.get(chunk_plan)
    if nc is None:
        nc = build_program(chunk_plan)
        _CACHE[chunk_plan] = nc
    res = run_bass_kernel_spmd(nc, in_maps, core_ids=list(range(8)))
    out = np.zeros((N, T * HID), np.float32)
    for c in range(8):
        q, r = c // P, c % P
        out[r * NQ:(r + 1) * NQ, q * HID:(q + 1) * HID] = \
            np.asarray(res.results[c]["out"], np.float32)[:NQ]
    return out

